# revision 9
# baseline (speedup 1.0000x reference)
"""CAPMemory loss kernel for 8 Trainium2 NeuronCores.

Sharding: camera-sharded -- core c owns memory[c], the batch is replicated
(the per-sample stats each core produces are tiny, so this moves 16x less
HBM traffic than batch-sharding the replicated 128 MiB memory bank).

Device, per core (fp8 e4m3 DoubleRow matmul, fp32 PSUM):
  S[b, l] = <x_norm[b], memory[c, l]> * FP8_SCALE^2      [1024, 2048]
  E       = exp(S / (FP8_SCALE^2 * T))  (ACT, bf16)
  zin[b]  = sum_l E[b, l]               (ACT free-dim accumulate, fp32)
  cand    = top-8 of each 256-wide chunk of E -> 64 values/sample (DVE MAX8)
Outputs ship as one [128, 65] f32 tile per batch-tile: 64 candidate
columns + the zin column.

Schedule (v2, from the baseline's NTFF profile):
  - input DMAs split into 24 x 256 KiB parts issued round-robin on the
    two HW DGE queues (SP + Activation) as the first user instructions,
    chunk-0 parts first -- the baseline serialized 16 issues on SP and
    the PE idled 12.9 us before its first real matmul;
  - a short bf16 warm-up matmul stream covers the chunk-0 DMA latency so
    the PE HAM clock-gate is released (~2.4 GHz) when real work arrives;
  - batch-tiles 0/1 accumulate into the two PSUM S-slots with their
    matmuls interleaved in chunk-arrival order during the fill;
  - tile 0's exp runs in two halves and tile 2's matmuls are bank-pair
    -major so the PE restarts ~0.3 us (not 1.4 us) after the fill;
  - the last tile is bank-pair-major with per-bank exp/MAX8 so only one
    bank's epilogue remains after its last matmul (baseline tail: 8.6 us).

Host merge:
  epos[c, b] = exp(<x8[b], m8[c, tgt_b]>/T') recomputed in f32 from the
  exact fp8 operands the device consumed; intra CE = log(zin) - log(epos)
  on the own-camera core. For the inter loss the positive's value is
  removed from its camera's candidate list (nearest match to epos), the
  8x64 candidates are merged, and the exact top-50 negatives feed the
  log-sum-exp. A global top-50 element can only be missing from the
  candidates if >=8 larger elements share its 256-chunk (P ~ 1e-5 per
  run, and the substitute is the next-ranked value, so the effect is
  ~1e-6 relative even then).
"""

import numpy as np

T = 0.05
HARD_NEG_K = 50
LOSS_WEIGHT = 0.5
N_CAMS = 8
L = 2048
D = 2048
B = 1024
NBT = 8          # batch tiles of 128
KC8 = 8          # contraction chunks of 256 (fp8 DoubleRow: 2 k-rows/cell)
FP8_SCALE = 32.0  # pre-scale before e4m3 cast (keeps values out of denormals)
NCH = 8          # candidate chunks per row
CHW = 256        # chunk width
NTOP = NCH * 8   # candidates shipped per camera (top-8 of each chunk)
N_WARM = 34      # warm-up matmuls covering chunk-0 DMA latency
NZC = 4          # zin partial columns (one per PSUM bank)

_CACHE = {}


def _split_multi_waits(nc):
    """This container's walrus build rejects instructions carrying more than
    one sync wait ('Too many sync wait commands'). Hoist all but the last
    wait of each instruction onto same-engine Drain carriers placed just
    before it — semantically identical on an in-order engine stream."""
    import concourse.mybir as mybir

    n = 0
    for fn in nc.m.functions:
        for bb in fn.blocks:
            out = []
            for inst in bb.instructions:
                si = inst.sync_info
                if si is not None and si.on_wait and len(si.on_wait) > 1:
                    waits = list(si.on_wait)
                    for w in waits[:-1]:
                        d = mybir.InstDrain(name=f"ws-{n}", ins=[], outs=[])
                        n += 1
                        d.engine = inst.engine
                        d.sync_info = mybir.SyncInfo(on_wait=[w], on_update=[])
                        out.append(d)
                    si.on_wait = [waits[-1]]
                out.append(inst)
            if n:
                bb.instructions = out


def _build():
    import concourse.bass as bass
    import concourse.mybir as mybir
    from concourse import tile

    f32 = mybir.dt.float32
    bf16 = mybir.dt.bfloat16
    f8 = mybir.dt.float8e4
    Act = mybir.ActivationFunctionType

    nc = bass.Bass()
    xT = nc.dram_tensor("xT", [KC8, 128, 2, B], f8, kind="ExternalInput")
    mT = nc.dram_tensor("mT", [KC8, 2, 128, 2, 1024], f8, kind="ExternalInput")
    res_d = nc.dram_tensor("res", [NBT, 128, NTOP + NZC], f32, kind="ExternalOutput")

    with tile.TileContext(nc) as tc:
        with (
            tc.tile_pool(name="const", bufs=1) as cpool,
            tc.tile_pool(name="psum", bufs=2, space="PSUM") as ppool,
            tc.tile_pool(name="work", bufs=3) as wpool,
            tc.tile_pool(name="small", bufs=3) as spool,
        ):
            X = cpool.tile([128, KC8, 2, B], f8)
            M = cpool.tile([128, KC8, 2, 2, 1024], f8)
            GB = cpool.tile([128, 128], bf16)

            # Input DMAs first, as 24 x 256 KiB parts alternated between the
            # two hardware DGE queues (SP=sync, Activation=scalar) so both
            # rings fill in parallel and chunk 0 lands ~3 us after the
            # preamble barrier instead of ~6 us.
            parts = []
            for kc in range(KC8):
                parts.append((X[:, kc], xT[kc]))
                parts.append((M[:, kc, 0], mT[kc, 0]))
                parts.append((M[:, kc, 1], mT[kc, 1]))
            for idx, (dst, src) in enumerate(parts):
                eng = nc.sync if idx % 2 == 0 else nc.scalar
                eng.dma_start(dst, src)

            # PE warm-up: HAM needs ~3.4us of sustained activity to release
            # the clock gate (1.2 -> 2.4 GHz). Run throwaway bf16 matmuls on
            # a small zeroed tile while the first input chunks are in
            # flight, sized to hand off to the real stream without a gap.
            nc.vector.memset(GB[:], 0.0)
            WARM = ppool.tile([128, 512], f32, tag="S0")
            for _ in range(N_WARM):
                nc.tensor.matmul(
                    WARM[:, 0:128], GB[:], GB[:], start=True, stop=True,
                )

            def rhs(kc, nch):
                l2, li = divmod(nch, 2)
                return M[:, kc, l2, :, li * 512 : (li + 1) * 512]

            def lhs(kc, bt):
                return X[:, kc, :, bt * 128 : (bt + 1) * 128]

            # One PSUM tile per bank (4 tags x 2 bufs = all 8 banks) so
            # cross-engine dependencies resolve per bank: exp(bank b) starts
            # as soon as bank b's accumulation stops instead of after the
            # btile's last matmul (PSUM tile deps are tracked whole-tile).
            def s_banks(bt):
                return [
                    ppool.tile([128, 512], f32, tag=f"S{b}", name=f"S_{bt}_{b}")
                    for b in range(4)
                ]

            escale = 1.0 / (FP8_SCALE * FP8_SCALE * T)

            def epilogue(S, bt, res):
                """Per-bank exp (+zin partial col) and candidate MAX8s.

                For the last btile, bank 3's candidates are taken by MAX8
                directly on the PSUM logits (DVE) in parallel with its zin
                exp (ACT), and the result DMA is split so only a 20-column
                sliver trails the last matmul; the host exp()s those
                logit-valued candidates back.
                """
                last = bt == NBT - 1
                E = wpool.tile([128, L], bf16, tag="E", name=f"E_{bt}")
                for b in range(4):
                    nc.scalar.activation(
                        E[:, b * 512 : (b + 1) * 512], S[b][:],
                        Act.Exp, scale=escale,
                        accum_out=res[:, NTOP + b : NTOP + b + 1],
                    )
                    if last and b == 3:
                        for ch in (6, 7):
                            nc.vector.max(
                                res[:, ch * 8 : (ch + 1) * 8],
                                S[3][:, (ch - 6) * CHW : (ch - 5) * CHW],
                            )
                    else:
                        for ch in (2 * b, 2 * b + 1):
                            nc.vector.max(
                                res[:, ch * 8 : (ch + 1) * 8],
                                E[:, ch * CHW : (ch + 1) * CHW],
                            )
                if last:
                    nc.sync.dma_start(res_d[bt][:, 0:48], res[:, 0:48])
                    nc.sync.dma_start(res_d[bt][:, 48:], res[:, 48:])
                else:
                    nc.sync.dma_start(res_d[bt], res[:])

            # btiles 0 and 1 accumulate into both PSUM slot-sets with their
            # matmuls interleaved in chunk-arrival order: each arriving
            # 256 KiB part enables its matmuls as soon as it lands.
            S0 = s_banks(0)
            S1 = s_banks(1)
            S_pair = [S0, S1]
            for i in range(KC8):
                for half in range(2):
                    for bt in range(2):
                        for nch in (2 * half, 2 * half + 1):
                            nc.tensor.matmul(
                                S_pair[bt][nch][:],
                                lhs(i, bt),
                                rhs(i, nch),
                                start=(i == 0),
                                stop=(i == KC8 - 1),
                                perf_mode=mybir.MatmulPerfMode.DoubleRow,
                            )

            res0 = spool.tile([128, NTOP + NZC], f32, tag="res")
            res1 = spool.tile([128, NTOP + NZC], f32, tag="res")
            epilogue(S0, 0, res0)
            epilogue(S1, 1, res1)

            for bt in range(2, NBT):
                S = s_banks(bt)
                bank_major = bt == 2 or bt == NBT - 1
                if bank_major:
                    # bank-major: bank b completes after 8 matmuls, so
                    # btile2 chases btile0's per-bank exps with no gap at
                    # the fill->steady transition, and btile7 leaves only
                    # bank 3's epilogue after the last matmul.
                    for nch in range(4):
                        for i in range(KC8):
                            nc.tensor.matmul(
                                S[nch][:],
                                lhs(i, bt),
                                rhs(i, nch),
                                start=(i == 0),
                                stop=(i == KC8 - 1),
                                perf_mode=mybir.MatmulPerfMode.DoubleRow,
                            )
                else:
                    for i in range(KC8):
                        for nch in range(4):
                            nc.tensor.matmul(
                                S[nch][:],
                                lhs(i, bt),
                                rhs(i, nch),
                                start=(i == 0),
                                stop=(i == KC8 - 1),
                                perf_mode=mybir.MatmulPerfMode.DoubleRow,
                            )
                res = spool.tile([128, NTOP + NZC], f32, tag="res", name=f"res_{bt}")
                epilogue(S, bt, res)

    _split_multi_waits(nc)
    return nc


def _get_nc():
    if "nc" not in _CACHE:
        _CACHE["nc"] = _build()
    return _CACHE["nc"]


def _pack_x(xT_f32, f8):
    # [D, B] -> [KC8, 128, 2, B] with d = kc*256 + j*128 + p
    v = np.clip(xT_f32 * FP8_SCALE, -240.0, 240.0)
    v = v.reshape(KC8, 2, 128, B).transpose(0, 2, 1, 3)
    return np.ascontiguousarray(v).astype(f8)


def _pack_m(mT_f32, f8):
    # [D, L] -> [KC8, 2, 128, 2, 1024]: (kc, l2, p, j, l1) with
    # d = kc*256 + j*128 + p and l = l2*1024 + l1
    v = np.clip(mT_f32 * FP8_SCALE, -240.0, 240.0)
    v = v.reshape(KC8, 2, 128, 2, 1024).transpose(0, 3, 2, 1, 4)
    return np.ascontiguousarray(v).astype(f8)


def _prepare_in_maps(inputs, memory):
    import ml_dtypes

    f8 = ml_dtypes.float8_e4m3
    inputs = np.asarray(inputs, np.float32)
    memory = np.asarray(memory, np.float32)
    x = inputs / np.linalg.norm(inputs, axis=1, keepdims=True)
    xT = _pack_x(x.T, f8)
    in_maps = []
    for c in range(N_CAMS):
        mT = _pack_m(memory[c].T, f8)
        in_maps.append({"xT": xT, "mT": mT})
    return in_maps


def kernel(inputs, memory, indexes, cams_all, labels_all):
    from concourse.bass_utils import run_bass_kernel_spmd

    indexes = np.asarray(indexes).astype(np.int64)
    cams_all = np.asarray(cams_all).astype(np.int64)
    labels_all = np.asarray(labels_all).astype(np.int64)
    cams = cams_all[indexes]

    in_maps = _prepare_in_maps(inputs, memory)
    nc = _get_nc()
    res = run_bass_kernel_spmd(nc, in_maps, list(range(N_CAMS)))

    # epos = exp(S[t]/T) computed host-side from the same fp8-quantized
    # inputs the device consumed (f32 arithmetic ~= PSUM fp32 accumulate)
    tgts = labels_all[indexes]
    x8 = in_maps[0]["xT"].transpose(0, 2, 1, 3).reshape(D, B).astype(np.float32)
    epos = np.empty((N_CAMS, B), np.float64)
    for c in range(N_CAMS):
        m8 = (
            in_maps[c]["mT"]
            .transpose(0, 3, 2, 1, 4)          # kc, j, p, l2, l1
            .reshape(D, L)
            .astype(np.float32)
        )
        mt = m8[:, tgts]                     # [D, B]
        s_t = np.einsum("db,db->b", x8, mt, optimize=True)
        epos[c] = np.exp(s_t.astype(np.float64) / (FP8_SCALE * FP8_SCALE * T))

    # gather per-core stats; [NBT, 128, 68] -> [B, 68] with b = bt*128 + p;
    # cols 64:68 are per-PSUM-bank zin partials. The last btile's cols
    # 48:64 are raw logits (MAX8 on PSUM) -> exp them here.
    escale = 1.0 / (FP8_SCALE * FP8_SCALE * T)
    zin = np.empty((N_CAMS, B), np.float64)
    topv = np.empty((N_CAMS, B, NTOP), np.float64)
    for c in range(N_CAMS):
        r = res.results[c]["res"].astype(np.float64).reshape(B, NTOP + NZC)
        topv[c] = r[:, :NTOP]
        topv[c, B - 128 :, 48:64] = np.exp(topv[c, B - 128 :, 48:64] * escale)
        zin[c] = r[:, NTOP:].sum(axis=1)

    # ---- intra: CE against own camera, mean within camera group, summed
    bidx = np.arange(B)
    zin_own = zin[cams, bidx]
    epos_own = epos[cams, bidx]
    ce = np.log(zin_own) - np.log(epos_own)
    cnt = np.bincount(cams, minlength=N_CAMS).astype(np.float64)
    ce_sum = np.bincount(cams, weights=ce, minlength=N_CAMS)
    loss_intra = np.sum(ce_sum / np.maximum(cnt, 1.0))

    # remove the positive's own value from each camera's candidate list:
    # nearest candidate within 0.5% of the host-computed epos (device values
    # are bf16-rounded, so exact equality is not available)
    for c in range(N_CAMS):
        relerr = np.abs(topv[c] - epos[c][:, None]) / epos[c][:, None]
        j = np.argmin(relerr, axis=1)
        hit = relerr[bidx, j] < 5e-3
        topv[c][bidx[hit], j[hit]] = 0.0

    # ---- inter: exact global top-50 negatives from 8x63 candidates
    cand = topv.transpose(1, 0, 2).reshape(B, N_CAMS * NTOP)
    part = np.partition(cand, cand.shape[1] - HARD_NEG_K, axis=1)
    z50 = part[:, cand.shape[1] - HARD_NEG_K :].sum(axis=1)
    sum_epos = epos.sum(axis=0)
    lse = np.log(sum_epos + z50)
    mean_logpos = np.log(epos).mean(axis=0)
    per_sample = lse - mean_logpos
    inter_sum = np.bincount(cams, weights=per_sample, minlength=N_CAMS)
    loss_inter = np.sum(inter_sum / np.maximum(cnt, 1.0)) * LOSS_WEIGHT

    return np.float32(loss_intra), np.float32(loss_inter)


# revision 10
# speedup vs baseline: 1.0149x; 1.0149x over previous
"""CAPMemory loss kernel for 8 Trainium2 NeuronCores.

Sharding: camera-sharded -- core c owns memory[c], the batch is replicated
(the per-sample stats each core produces are tiny, so this moves 16x less
HBM traffic than batch-sharding the replicated 128 MiB memory bank).

Device, per core (fp8 e4m3 DoubleRow matmul, fp32 PSUM):
  S[b, l] = <x_norm[b], memory[c, l]> * FP8_SCALE^2      [1024, 2048]
  E       = exp(S / (FP8_SCALE^2 * T))  (ACT, bf16)
  zin[b]  = sum_l E[b, l]               (ACT free-dim accumulate, fp32)
  cand    = top-8 of each 256-wide chunk of E -> 64 values/sample (DVE MAX8)
Outputs ship as one [128, 65] f32 tile per batch-tile: 64 candidate
columns + the zin column.

Schedule (v2, from the baseline's NTFF profile):
  - input DMAs split into 24 x 256 KiB parts issued round-robin on the
    two HW DGE queues (SP + Activation) as the first user instructions,
    chunk-0 parts first -- the baseline serialized 16 issues on SP and
    the PE idled 12.9 us before its first real matmul;
  - a short bf16 warm-up matmul stream covers the chunk-0 DMA latency so
    the PE HAM clock-gate is released (~2.4 GHz) when real work arrives;
  - batch-tiles 0/1 accumulate into the two PSUM S-slots with their
    matmuls interleaved in chunk-arrival order during the fill;
  - tile 0's exp runs in two halves and tile 2's matmuls are bank-pair
    -major so the PE restarts ~0.3 us (not 1.4 us) after the fill;
  - the last tile is bank-pair-major with per-bank exp/MAX8 so only one
    bank's epilogue remains after its last matmul (baseline tail: 8.6 us).

Host merge:
  epos[c, b] = exp(<x8[b], m8[c, tgt_b]>/T') recomputed in f32 from the
  exact fp8 operands the device consumed; intra CE = log(zin) - log(epos)
  on the own-camera core. For the inter loss the positive's value is
  removed from its camera's candidate list (nearest match to epos), the
  8x64 candidates are merged, and the exact top-50 negatives feed the
  log-sum-exp. A global top-50 element can only be missing from the
  candidates if >=8 larger elements share its 256-chunk (P ~ 1e-5 per
  run, and the substitute is the next-ranked value, so the effect is
  ~1e-6 relative even then).
"""

import numpy as np

T = 0.05
HARD_NEG_K = 50
LOSS_WEIGHT = 0.5
N_CAMS = 8
L = 2048
D = 2048
B = 1024
NBT = 8          # batch tiles of 128
KC8 = 8          # contraction chunks of 256 (fp8 DoubleRow: 2 k-rows/cell)
FP8_SCALE = 32.0  # pre-scale before e4m3 cast (keeps values out of denormals)
NCH = 8          # candidate chunks per row
CHW = 256        # chunk width
NTOP = NCH * 8   # candidates shipped per camera (top-8 of each chunk)
N_WARM = 34      # warm-up matmuls covering chunk-0 DMA latency
NZC = 4          # zin partial columns (one per PSUM bank)

_CACHE = {}


def _split_multi_waits(nc):
    """This container's walrus build rejects instructions carrying more than
    one sync wait ('Too many sync wait commands'). Hoist all but the last
    wait of each instruction onto same-engine Drain carriers placed just
    before it — semantically identical on an in-order engine stream."""
    import concourse.mybir as mybir

    n = 0
    for fn in nc.m.functions:
        for bb in fn.blocks:
            out = []
            for inst in bb.instructions:
                si = inst.sync_info
                if si is not None and si.on_wait and len(si.on_wait) > 1:
                    waits = list(si.on_wait)
                    for w in waits[:-1]:
                        d = mybir.InstDrain(name=f"ws-{n}", ins=[], outs=[])
                        n += 1
                        d.engine = inst.engine
                        d.sync_info = mybir.SyncInfo(on_wait=[w], on_update=[])
                        out.append(d)
                    si.on_wait = [waits[-1]]
                out.append(inst)
            if n:
                bb.instructions = out


def _build():
    import concourse.bass as bass
    import concourse.mybir as mybir
    from concourse import tile

    f32 = mybir.dt.float32
    bf16 = mybir.dt.bfloat16
    f8 = mybir.dt.float8e4
    Act = mybir.ActivationFunctionType

    nc = bass.Bass()
    xT = nc.dram_tensor("xT", [KC8, 128, 2, B], f8, kind="ExternalInput")
    mT = nc.dram_tensor("mT", [KC8, 2, 128, 2, 1024], f8, kind="ExternalInput")
    res_d = nc.dram_tensor("res", [NBT, 128, NTOP + NZC], f32, kind="ExternalOutput")

    with tile.TileContext(nc) as tc:
        with (
            tc.tile_pool(name="const", bufs=1) as cpool,
            tc.tile_pool(name="psum", bufs=2, space="PSUM") as ppool,
            tc.tile_pool(name="work", bufs=3) as wpool,
            tc.tile_pool(name="small", bufs=3) as spool,
        ):
            X = cpool.tile([128, KC8, 2, B], f8)
            M = cpool.tile([128, KC8, 2, 2, 1024], f8)
            GB = cpool.tile([128, 128], bf16)

            # Input DMAs first, as 24 x 256 KiB parts alternated between the
            # two hardware DGE queues (SP=sync, Activation=scalar) so both
            # rings fill in parallel and chunk 0 lands ~3 us after the
            # preamble barrier instead of ~6 us.
            parts = []
            for kc in range(KC8):
                parts.append((X[:, kc], xT[kc]))
                parts.append((M[:, kc, 0], mT[kc, 0]))
                parts.append((M[:, kc, 1], mT[kc, 1]))
            for idx, (dst, src) in enumerate(parts):
                eng = nc.sync if idx % 2 == 0 else nc.scalar
                eng.dma_start(dst, src)

            # PE warm-up: HAM needs ~3.4us of sustained activity to release
            # the clock gate (1.2 -> 2.4 GHz). Run throwaway bf16 matmuls on
            # a small zeroed tile while the first input chunks are in
            # flight, sized to hand off to the real stream without a gap.
            nc.vector.memset(GB[:], 0.0)
            WARM = ppool.tile([128, 512], f32, tag="S0")
            for _ in range(N_WARM):
                nc.tensor.matmul(
                    WARM[:, 0:128], GB[:], GB[:], start=True, stop=True,
                )

            def rhs(kc, nch):
                l2, li = divmod(nch, 2)
                return M[:, kc, l2, :, li * 512 : (li + 1) * 512]

            def lhs(kc, bt):
                return X[:, kc, :, bt * 128 : (bt + 1) * 128]

            # One PSUM tile per bank (4 tags x 2 bufs = all 8 banks) so
            # cross-engine dependencies resolve per bank: exp(bank b) starts
            # as soon as bank b's accumulation stops instead of after the
            # btile's last matmul (PSUM tile deps are tracked whole-tile).
            def s_banks(bt):
                return [
                    ppool.tile([128, 512], f32, tag=f"S{b}", name=f"S_{bt}_{b}")
                    for b in range(4)
                ]

            escale = 1.0 / (FP8_SCALE * FP8_SCALE * T)

            def epilogue(S, bt, res):
                """Per-bank exp (+zin partial col) and candidate MAX8s.

                For the last btile, bank 3's candidates are taken by MAX8
                directly on the PSUM logits (DVE) in parallel with its zin
                exp (ACT), and the result DMA is split so only a 20-column
                sliver trails the last matmul; the host exp()s those
                logit-valued candidates back.
                """
                last = bt == NBT - 1
                E = wpool.tile([128, L], bf16, tag="E", name=f"E_{bt}")
                for b in range(4):
                    if last and b == 3:
                        # zin partial into a separate tile so the MAX8s
                        # (which write res) don't serialize behind the
                        # accumulator read (write-write tile dep).
                        ZL = spool.tile([128, 1], f32, tag="zl")
                        nc.scalar.activation(
                            E[:, b * 512 : (b + 1) * 512], S[b][:],
                            Act.Exp, scale=escale, accum_out=ZL[:],
                        )
                        for ch in (6, 7):
                            nc.vector.max(
                                res[:, ch * 8 : (ch + 1) * 8],
                                S[3][:, (ch - 6) * CHW : (ch - 5) * CHW],
                            )
                        nc.scalar.dma_start(
                            res_d[bt][:, NTOP + 3 : NTOP + 4], ZL[:]
                        )
                    else:
                        nc.scalar.activation(
                            E[:, b * 512 : (b + 1) * 512], S[b][:],
                            Act.Exp, scale=escale,
                            accum_out=res[:, NTOP + b : NTOP + b + 1],
                        )
                        for ch in (2 * b, 2 * b + 1):
                            nc.vector.max(
                                res[:, ch * 8 : (ch + 1) * 8],
                                E[:, ch * CHW : (ch + 1) * CHW],
                            )
                if last:
                    nc.sync.dma_start(res_d[bt][:, 0:48], res[:, 0:48])
                    nc.sync.dma_start(
                        res_d[bt][:, 48 : NTOP + 3], res[:, 48 : NTOP + 3]
                    )
                else:
                    nc.sync.dma_start(res_d[bt], res[:])

            # btiles 0 and 1 accumulate into both PSUM slot-sets with their
            # matmuls interleaved in chunk-arrival order: each arriving
            # 256 KiB part enables its matmuls as soon as it lands.
            S0 = s_banks(0)
            S1 = s_banks(1)
            S_pair = [S0, S1]
            for i in range(KC8):
                for half in range(2):
                    for bt in range(2):
                        for nch in (2 * half, 2 * half + 1):
                            nc.tensor.matmul(
                                S_pair[bt][nch][:],
                                lhs(i, bt),
                                rhs(i, nch),
                                start=(i == 0),
                                stop=(i == KC8 - 1),
                                perf_mode=mybir.MatmulPerfMode.DoubleRow,
                            )

            res0 = spool.tile([128, NTOP + NZC], f32, tag="res")
            res1 = spool.tile([128, NTOP + NZC], f32, tag="res")
            epilogue(S0, 0, res0)
            epilogue(S1, 1, res1)

            for bt in range(2, NBT):
                S = s_banks(bt)
                bank_major = bt == 2 or bt == NBT - 1
                if bank_major:
                    # bank-major: bank b completes after 8 matmuls, so
                    # btile2 chases btile0's per-bank exps with no gap at
                    # the fill->steady transition, and btile7 leaves only
                    # bank 3's epilogue after the last matmul.
                    for nch in range(4):
                        for i in range(KC8):
                            nc.tensor.matmul(
                                S[nch][:],
                                lhs(i, bt),
                                rhs(i, nch),
                                start=(i == 0),
                                stop=(i == KC8 - 1),
                                perf_mode=mybir.MatmulPerfMode.DoubleRow,
                            )
                else:
                    for i in range(KC8):
                        for nch in range(4):
                            nc.tensor.matmul(
                                S[nch][:],
                                lhs(i, bt),
                                rhs(i, nch),
                                start=(i == 0),
                                stop=(i == KC8 - 1),
                                perf_mode=mybir.MatmulPerfMode.DoubleRow,
                            )
                res = spool.tile([128, NTOP + NZC], f32, tag="res", name=f"res_{bt}")
                epilogue(S, bt, res)

    _split_multi_waits(nc)
    return nc


def _get_nc():
    if "nc" not in _CACHE:
        _CACHE["nc"] = _build()
    return _CACHE["nc"]


def _pack_x(xT_f32, f8):
    # [D, B] -> [KC8, 128, 2, B] with d = kc*256 + j*128 + p
    v = np.clip(xT_f32 * FP8_SCALE, -240.0, 240.0)
    v = v.reshape(KC8, 2, 128, B).transpose(0, 2, 1, 3)
    return np.ascontiguousarray(v).astype(f8)


def _pack_m(mT_f32, f8):
    # [D, L] -> [KC8, 2, 128, 2, 1024]: (kc, l2, p, j, l1) with
    # d = kc*256 + j*128 + p and l = l2*1024 + l1
    v = np.clip(mT_f32 * FP8_SCALE, -240.0, 240.0)
    v = v.reshape(KC8, 2, 128, 2, 1024).transpose(0, 3, 2, 1, 4)
    return np.ascontiguousarray(v).astype(f8)


def _prepare_in_maps(inputs, memory):
    import ml_dtypes

    f8 = ml_dtypes.float8_e4m3
    inputs = np.asarray(inputs, np.float32)
    memory = np.asarray(memory, np.float32)
    x = inputs / np.linalg.norm(inputs, axis=1, keepdims=True)
    xT = _pack_x(x.T, f8)
    in_maps = []
    for c in range(N_CAMS):
        mT = _pack_m(memory[c].T, f8)
        in_maps.append({"xT": xT, "mT": mT})
    return in_maps


def kernel(inputs, memory, indexes, cams_all, labels_all):
    from concourse.bass_utils import run_bass_kernel_spmd

    indexes = np.asarray(indexes).astype(np.int64)
    cams_all = np.asarray(cams_all).astype(np.int64)
    labels_all = np.asarray(labels_all).astype(np.int64)
    cams = cams_all[indexes]

    in_maps = _prepare_in_maps(inputs, memory)
    nc = _get_nc()
    res = run_bass_kernel_spmd(nc, in_maps, list(range(N_CAMS)))

    # epos = exp(S[t]/T) computed host-side from the same fp8-quantized
    # inputs the device consumed (f32 arithmetic ~= PSUM fp32 accumulate)
    tgts = labels_all[indexes]
    x8 = in_maps[0]["xT"].transpose(0, 2, 1, 3).reshape(D, B).astype(np.float32)
    epos = np.empty((N_CAMS, B), np.float64)
    for c in range(N_CAMS):
        m8 = (
            in_maps[c]["mT"]
            .transpose(0, 3, 2, 1, 4)          # kc, j, p, l2, l1
            .reshape(D, L)
            .astype(np.float32)
        )
        mt = m8[:, tgts]                     # [D, B]
        s_t = np.einsum("db,db->b", x8, mt, optimize=True)
        epos[c] = np.exp(s_t.astype(np.float64) / (FP8_SCALE * FP8_SCALE * T))

    # gather per-core stats; [NBT, 128, 68] -> [B, 68] with b = bt*128 + p;
    # cols 64:68 are per-PSUM-bank zin partials. The last btile's cols
    # 48:64 are raw logits (MAX8 on PSUM) -> exp them here.
    escale = 1.0 / (FP8_SCALE * FP8_SCALE * T)
    zin = np.empty((N_CAMS, B), np.float64)
    topv = np.empty((N_CAMS, B, NTOP), np.float64)
    for c in range(N_CAMS):
        r = res.results[c]["res"].astype(np.float64).reshape(B, NTOP + NZC)
        topv[c] = r[:, :NTOP]
        topv[c, B - 128 :, 48:64] = np.exp(topv[c, B - 128 :, 48:64] * escale)
        zin[c] = r[:, NTOP:].sum(axis=1)

    # ---- intra: CE against own camera, mean within camera group, summed
    bidx = np.arange(B)
    zin_own = zin[cams, bidx]
    epos_own = epos[cams, bidx]
    ce = np.log(zin_own) - np.log(epos_own)
    cnt = np.bincount(cams, minlength=N_CAMS).astype(np.float64)
    ce_sum = np.bincount(cams, weights=ce, minlength=N_CAMS)
    loss_intra = np.sum(ce_sum / np.maximum(cnt, 1.0))

    # remove the positive's own value from each camera's candidate list:
    # nearest candidate within 0.5% of the host-computed epos (device values
    # are bf16-rounded, so exact equality is not available)
    for c in range(N_CAMS):
        relerr = np.abs(topv[c] - epos[c][:, None]) / epos[c][:, None]
        j = np.argmin(relerr, axis=1)
        hit = relerr[bidx, j] < 5e-3
        topv[c][bidx[hit], j[hit]] = 0.0

    # ---- inter: exact global top-50 negatives from 8x63 candidates
    cand = topv.transpose(1, 0, 2).reshape(B, N_CAMS * NTOP)
    part = np.partition(cand, cand.shape[1] - HARD_NEG_K, axis=1)
    z50 = part[:, cand.shape[1] - HARD_NEG_K :].sum(axis=1)
    sum_epos = epos.sum(axis=0)
    lse = np.log(sum_epos + z50)
    mean_logpos = np.log(epos).mean(axis=0)
    per_sample = lse - mean_logpos
    inter_sum = np.bincount(cams, weights=per_sample, minlength=N_CAMS)
    loss_inter = np.sum(inter_sum / np.maximum(cnt, 1.0)) * LOSS_WEIGHT

    return np.float32(loss_intra), np.float32(loss_inter)


# revision 12
# speedup vs baseline: 1.0344x; 1.0192x over previous
"""CAPMemory loss kernel for 8 Trainium2 NeuronCores.

Sharding: camera-sharded -- core c owns memory[c], the batch is replicated
(the per-sample stats each core produces are tiny, so this moves 16x less
HBM traffic than batch-sharding the replicated 128 MiB memory bank).

Device, per core (fp8 e4m3 DoubleRow matmul, fp32 PSUM):
  S[b, l] = <x_norm[b], memory[c, l]> * FP8_SCALE^2      [1024, 2048]
  E       = exp(S / (FP8_SCALE^2 * T))  (ACT, bf16)
  zin[b]  = sum_l E[b, l]               (ACT free-dim accumulate, fp32)
  cand    = top-8 of each 256-wide chunk of E -> 64 values/sample (DVE MAX8)
Outputs ship as one [128, 65] f32 tile per batch-tile: 64 candidate
columns + the zin column.

Schedule (v2, from the baseline's NTFF profile):
  - input DMAs split into 24 x 256 KiB parts issued round-robin on the
    two HW DGE queues (SP + Activation) as the first user instructions,
    chunk-0 parts first -- the baseline serialized 16 issues on SP and
    the PE idled 12.9 us before its first real matmul;
  - a short bf16 warm-up matmul stream covers the chunk-0 DMA latency so
    the PE HAM clock-gate is released (~2.4 GHz) when real work arrives;
  - batch-tiles 0/1 accumulate into the two PSUM S-slots with their
    matmuls interleaved in chunk-arrival order during the fill;
  - tile 0's exp runs in two halves and tile 2's matmuls are bank-pair
    -major so the PE restarts ~0.3 us (not 1.4 us) after the fill;
  - the last tile is bank-pair-major with per-bank exp/MAX8 so only one
    bank's epilogue remains after its last matmul (baseline tail: 8.6 us).

Host merge:
  epos[c, b] = exp(<x8[b], m8[c, tgt_b]>/T') recomputed in f32 from the
  exact fp8 operands the device consumed; intra CE = log(zin) - log(epos)
  on the own-camera core. For the inter loss the positive's value is
  removed from its camera's candidate list (nearest match to epos), the
  8x64 candidates are merged, and the exact top-50 negatives feed the
  log-sum-exp. A global top-50 element can only be missing from the
  candidates if >=8 larger elements share its 256-chunk (P ~ 1e-5 per
  run, and the substitute is the next-ranked value, so the effect is
  ~1e-6 relative even then).
"""

import numpy as np

T = 0.05
HARD_NEG_K = 50
LOSS_WEIGHT = 0.5
N_CAMS = 8
L = 2048
D = 2048
B = 1024
NBT = 8          # batch tiles of 128
KC8 = 8          # contraction chunks of 256 (fp8 DoubleRow: 2 k-rows/cell)
FP8_SCALE = 32.0  # pre-scale before e4m3 cast (keeps values out of denormals)
NCH = 8          # candidate chunks per row
CHW = 256        # chunk width
NTOP = NCH * 8   # candidates shipped per camera (top-8 of each chunk)
N_WARM = 34      # warm-up matmuls covering chunk-0 DMA latency
NZC = 4          # zin partial columns (one per PSUM bank)

_CACHE = {}


def _split_multi_waits(nc):
    """This container's walrus build rejects instructions carrying more than
    one sync wait ('Too many sync wait commands'). Hoist all but the last
    wait of each instruction onto same-engine Drain carriers placed just
    before it — semantically identical on an in-order engine stream."""
    import concourse.mybir as mybir

    n = 0
    for fn in nc.m.functions:
        for bb in fn.blocks:
            out = []
            for inst in bb.instructions:
                si = inst.sync_info
                if si is not None and si.on_wait and len(si.on_wait) > 1:
                    waits = list(si.on_wait)
                    for w in waits[:-1]:
                        d = mybir.InstDrain(name=f"ws-{n}", ins=[], outs=[])
                        n += 1
                        d.engine = inst.engine
                        d.sync_info = mybir.SyncInfo(on_wait=[w], on_update=[])
                        out.append(d)
                    si.on_wait = [waits[-1]]
                out.append(inst)
            if n:
                bb.instructions = out


def _build():
    import concourse.bass as bass
    import concourse.mybir as mybir
    from concourse import tile

    f32 = mybir.dt.float32
    bf16 = mybir.dt.bfloat16
    f8 = mybir.dt.float8e4
    Act = mybir.ActivationFunctionType

    nc = bass.Bass()
    xT = nc.dram_tensor("xT", [KC8, 128, 2, B], f8, kind="ExternalInput")
    mT = nc.dram_tensor("mT", [KC8, 2, 128, 2, 1024], f8, kind="ExternalInput")
    res_d = nc.dram_tensor("res", [NBT, 128, NTOP + NZC], f32, kind="ExternalOutput")

    with tile.TileContext(nc) as tc:
        with (
            tc.tile_pool(name="const", bufs=1) as cpool,
            tc.tile_pool(name="psum", bufs=2, space="PSUM") as ppool,
            tc.tile_pool(name="work", bufs=3) as wpool,
            tc.tile_pool(name="small", bufs=3) as spool,
        ):
            X = cpool.tile([128, KC8, 2, B], f8)
            M = cpool.tile([128, KC8, 2, 2, 1024], f8)
            GB = cpool.tile([128, 128], bf16)

            # Input DMAs first, as 24 x 256 KiB parts alternated between the
            # two hardware DGE queues (SP=sync, Activation=scalar) so both
            # rings fill in parallel and chunk 0 lands ~3 us after the
            # preamble barrier instead of ~6 us.
            parts = []
            for kc in range(KC8):
                parts.append((X[:, kc], xT[kc]))
                parts.append((M[:, kc, 0], mT[kc, 0]))
                parts.append((M[:, kc, 1], mT[kc, 1]))
            for idx, (dst, src) in enumerate(parts):
                eng = nc.sync if idx % 2 == 0 else nc.scalar
                eng.dma_start(dst, src)

            # PE warm-up: HAM needs ~3.4us of sustained activity to release
            # the clock gate (1.2 -> 2.4 GHz). Run throwaway bf16 matmuls on
            # a small zeroed tile while the first input chunks are in
            # flight, sized to hand off to the real stream without a gap.
            nc.vector.memset(GB[:], 0.0)
            WARM = ppool.tile([128, 512], f32, tag="S0")
            for _ in range(N_WARM):
                nc.tensor.matmul(
                    WARM[:, 0:128], GB[:], GB[:], start=True, stop=True,
                )

            def rhs(kc, nch):
                l2, li = divmod(nch, 2)
                return M[:, kc, l2, :, li * 512 : (li + 1) * 512]

            def lhs(kc, bt):
                return X[:, kc, :, bt * 128 : (bt + 1) * 128]

            # One PSUM tile per bank (4 tags x 2 bufs = all 8 banks) so
            # cross-engine dependencies resolve per bank: exp(bank b) starts
            # as soon as bank b's accumulation stops instead of after the
            # btile's last matmul (PSUM tile deps are tracked whole-tile).
            def s_banks(bt):
                return [
                    ppool.tile([128, 512], f32, tag=f"S{b}", name=f"S_{bt}_{b}")
                    for b in range(4)
                ]

            escale = 1.0 / (FP8_SCALE * FP8_SCALE * T)

            def epilogue(S, bt, res):
                """Per-bank exp (+zin partial col) and candidate MAX8s.

                For the last btile, bank 3's candidates are taken by MAX8
                directly on the PSUM logits (DVE) in parallel with its zin
                exp (ACT), and the result DMA is split so only a 20-column
                sliver trails the last matmul; the host exp()s those
                logit-valued candidates back.
                """
                last = bt == NBT - 1
                E = wpool.tile([128, L], bf16, tag="E", name=f"E_{bt}")
                for b in range(4):
                    nc.scalar.activation(
                        E[:, b * 512 : (b + 1) * 512], S[b][:],
                        Act.Exp, scale=escale,
                        accum_out=res[:, NTOP + b : NTOP + b + 1],
                    )
                    for ch in (2 * b, 2 * b + 1):
                        nc.vector.max(
                            res[:, ch * 8 : (ch + 1) * 8],
                            E[:, ch * CHW : (ch + 1) * CHW],
                        )
                if last:
                    # ship everything that's ready before bank 3's epilogue
                    # so only a 20-column sliver trails the last matmul
                    nc.sync.dma_start(res_d[bt][:, 0:48], res[:, 0:48])
                    nc.sync.dma_start(res_d[bt][:, 48:], res[:, 48:])
                else:
                    nc.sync.dma_start(res_d[bt], res[:])

            # btiles 0 and 1 accumulate into both PSUM slot-sets with their
            # matmuls interleaved in chunk-arrival order: each arriving
            # 256 KiB part enables its matmuls as soon as it lands.
            S0 = s_banks(0)
            S1 = s_banks(1)
            S_pair = [S0, S1]
            for i in range(KC8):
                for half in range(2):
                    for bt in range(2):
                        for nch in (2 * half, 2 * half + 1):
                            nc.tensor.matmul(
                                S_pair[bt][nch][:],
                                lhs(i, bt),
                                rhs(i, nch),
                                start=(i == 0),
                                stop=(i == KC8 - 1),
                                perf_mode=mybir.MatmulPerfMode.DoubleRow,
                            )

            res0 = spool.tile([128, NTOP + NZC], f32, tag="res")
            res1 = spool.tile([128, NTOP + NZC], f32, tag="res")
            epilogue(S0, 0, res0)
            epilogue(S1, 1, res1)

            for bt in range(2, NBT):
                S = s_banks(bt)
                bank_major = bt == 2 or bt == NBT - 1
                if bank_major:
                    # bank-major: bank b completes after 8 matmuls, so
                    # btile2 chases btile0's per-bank exps with no gap at
                    # the fill->steady transition, and btile7 leaves only
                    # bank 3's epilogue after the last matmul.
                    for nch in range(4):
                        for i in range(KC8):
                            nc.tensor.matmul(
                                S[nch][:],
                                lhs(i, bt),
                                rhs(i, nch),
                                start=(i == 0),
                                stop=(i == KC8 - 1),
                                perf_mode=mybir.MatmulPerfMode.DoubleRow,
                            )
                else:
                    for i in range(KC8):
                        for nch in range(4):
                            nc.tensor.matmul(
                                S[nch][:],
                                lhs(i, bt),
                                rhs(i, nch),
                                start=(i == 0),
                                stop=(i == KC8 - 1),
                                perf_mode=mybir.MatmulPerfMode.DoubleRow,
                            )
                res = spool.tile([128, NTOP + NZC], f32, tag="res", name=f"res_{bt}")
                epilogue(S, bt, res)

    _split_multi_waits(nc)
    return nc


def _get_nc():
    if "nc" not in _CACHE:
        _CACHE["nc"] = _build()
    return _CACHE["nc"]


def _pack_x(xT_f32, f8):
    # [D, B] -> [KC8, 128, 2, B] with d = kc*256 + j*128 + p
    v = np.clip(xT_f32 * FP8_SCALE, -240.0, 240.0)
    v = v.reshape(KC8, 2, 128, B).transpose(0, 2, 1, 3)
    return np.ascontiguousarray(v).astype(f8)


def _pack_m(mT_f32, f8):
    # [D, L] -> [KC8, 2, 128, 2, 1024]: (kc, l2, p, j, l1) with
    # d = kc*256 + j*128 + p and l = l2*1024 + l1
    v = np.clip(mT_f32 * FP8_SCALE, -240.0, 240.0)
    v = v.reshape(KC8, 2, 128, 2, 1024).transpose(0, 3, 2, 1, 4)
    return np.ascontiguousarray(v).astype(f8)


def _prepare_in_maps(inputs, memory):
    import ml_dtypes

    f8 = ml_dtypes.float8_e4m3
    inputs = np.asarray(inputs, np.float32)
    memory = np.asarray(memory, np.float32)
    x = inputs / np.linalg.norm(inputs, axis=1, keepdims=True)
    xT = _pack_x(x.T, f8)
    in_maps = []
    for c in range(N_CAMS):
        mT = _pack_m(memory[c].T, f8)
        in_maps.append({"xT": xT, "mT": mT})
    return in_maps


def kernel(inputs, memory, indexes, cams_all, labels_all):
    from concourse.bass_utils import run_bass_kernel_spmd

    indexes = np.asarray(indexes).astype(np.int64)
    cams_all = np.asarray(cams_all).astype(np.int64)
    labels_all = np.asarray(labels_all).astype(np.int64)
    cams = cams_all[indexes]

    in_maps = _prepare_in_maps(inputs, memory)
    nc = _get_nc()
    res = run_bass_kernel_spmd(nc, in_maps, list(range(N_CAMS)))

    # epos = exp(S[t]/T) computed host-side from the same fp8-quantized
    # inputs the device consumed (f32 arithmetic ~= PSUM fp32 accumulate)
    tgts = labels_all[indexes]
    x8 = in_maps[0]["xT"].transpose(0, 2, 1, 3).reshape(D, B).astype(np.float32)
    epos = np.empty((N_CAMS, B), np.float64)
    for c in range(N_CAMS):
        m8 = (
            in_maps[c]["mT"]
            .transpose(0, 3, 2, 1, 4)          # kc, j, p, l2, l1
            .reshape(D, L)
            .astype(np.float32)
        )
        mt = m8[:, tgts]                     # [D, B]
        s_t = np.einsum("db,db->b", x8, mt, optimize=True)
        epos[c] = np.exp(s_t.astype(np.float64) / (FP8_SCALE * FP8_SCALE * T))

    # gather per-core stats; [NBT, 128, 68] -> [B, 68] with b = bt*128 + p;
    # cols 64:68 are per-PSUM-bank zin partials
    zin = np.empty((N_CAMS, B), np.float64)
    topv = np.empty((N_CAMS, B, NTOP), np.float64)
    for c in range(N_CAMS):
        r = res.results[c]["res"].astype(np.float64).reshape(B, NTOP + NZC)
        topv[c] = r[:, :NTOP]
        zin[c] = r[:, NTOP:].sum(axis=1)

    # ---- intra: CE against own camera, mean within camera group, summed
    bidx = np.arange(B)
    zin_own = zin[cams, bidx]
    epos_own = epos[cams, bidx]
    ce = np.log(zin_own) - np.log(epos_own)
    cnt = np.bincount(cams, minlength=N_CAMS).astype(np.float64)
    ce_sum = np.bincount(cams, weights=ce, minlength=N_CAMS)
    loss_intra = np.sum(ce_sum / np.maximum(cnt, 1.0))

    # remove the positive's own value from each camera's candidate list:
    # nearest candidate within 0.5% of the host-computed epos (device values
    # are bf16-rounded, so exact equality is not available)
    for c in range(N_CAMS):
        relerr = np.abs(topv[c] - epos[c][:, None]) / epos[c][:, None]
        j = np.argmin(relerr, axis=1)
        hit = relerr[bidx, j] < 5e-3
        topv[c][bidx[hit], j[hit]] = 0.0

    # ---- inter: exact global top-50 negatives from 8x63 candidates
    cand = topv.transpose(1, 0, 2).reshape(B, N_CAMS * NTOP)
    part = np.partition(cand, cand.shape[1] - HARD_NEG_K, axis=1)
    z50 = part[:, cand.shape[1] - HARD_NEG_K :].sum(axis=1)
    sum_epos = epos.sum(axis=0)
    lse = np.log(sum_epos + z50)
    mean_logpos = np.log(epos).mean(axis=0)
    per_sample = lse - mean_logpos
    inter_sum = np.bincount(cams, weights=per_sample, minlength=N_CAMS)
    loss_inter = np.sum(inter_sum / np.maximum(cnt, 1.0)) * LOSS_WEIGHT

    return np.float32(loss_intra), np.float32(loss_inter)
